# revision 11
# baseline (speedup 1.0000x reference)
"""GateLoop fused Bass/Tile kernel for Trainium2, SPMD over 8 NeuronCores.

Problem (B=2, S=4096, D=1024):
    xn = rmsnorm(x) * gamma * sqrt(D)         (sum-of-squares norm)
    q,k,v = xn@wq, xn@wk, xn@wv ; a = sigmoid(xn@wa) ; g = xn@wg
    s_t = a_t * s_{t-1} + (k_t*v_t)           (elementwise linear recurrence)
    out = (q*s * silu(g)) @ wo
a
Sharding: sequence-parallel. Core c handles batch c//4, tokens
[(c%4)*1024, (c%4+1)*1024). The cross-chunk scan carry is resolved with the
decomposition  s = s_local + cumA * s_in:  each core computes per-chunk
summaries (A_total, s_last), AllGathers them (8KB), combines prefixes
locally, and applies its incoming state as a per-channel scalar.

Schedule: the AllGather's completion is gated by the slowest core's arrival
(launch skew across the 8 PJRT devices is ~90us), so the kernel is ordered
to bank carry-independent work behind the collective: (k,v,a) projections +
local scans first -> single AllGather of both summary halves -> (g,q)
projections + wo prefetch while the gather is in flight -> only the carry
apply + output projection remain on the dependent tail.

gamma is folded into the five input-side projection weights on the host.
Matmuls run in fp16 at full (double-pumped) PE rate.
"""

import numpy as np

import concourse.bacc as bacc
import concourse.tile as tile
from concourse import mybir
from concourse.bass_utils import run_bass_kernel_spmd
from concourse.masks import make_identity

AFT = mybir.ActivationFunctionType
ALU = mybir.AluOpType
F32 = mybir.dt.float32
F16 = mybir.dt.float16

B, S, D = 2, 4096, 1024
NCORE = 8
GROUPS = 2              # batch groups of 4 cores
CPG = NCORE // GROUPS   # chunks (cores) per group
CHUNK = (B * S) // NCORE  # 1024 tokens per core
P = 128
NPT = D // P            # 8 channel ptiles
NM = CHUNK // P         # 8 token tiles
H = 512                 # psum half width (fp32 bank)
EPS = 1e-5
YSC = 4096.0            # fp16 range guard: y is carried as y/YSC
SW = 2 * NPT            # summary width: [A_total | s_last] per ptile

_CACHE = {}


def _build():
    nc = bacc.Bacc("TRN2", target_bir_lowering=False, debug=False,
                   num_devices=NCORE)
    x_in = nc.dram_tensor("x", [CHUNK, D], F32, kind="ExternalInput")
    w_in = {
        n: nc.dram_tensor(n, [NPT, P, D], F16, kind="ExternalInput")
        for n in ("wg", "wq", "wk", "wv", "wa")
    }
    w_in["wo"] = nc.dram_tensor("wo", [2, NPT, P, H], F16,
                                kind="ExternalInput")
    mask_in = nc.dram_tensor("mask", [P, NPT * NCORE], F32,
                             kind="ExternalInput")
    out_t = nc.dram_tensor("out", [CHUNK, D], F32, kind="ExternalOutput")

    with tc_ctx(nc) as tc:
        with (
            tc.tile_pool(name="const", bufs=1) as const,
            tc.tile_pool(name="xny", bufs=NPT) as xny,
            tc.tile_pool(name="persist", bufs=1) as persist,
            tc.tile_pool(name="scr", bufs=2) as scr,
            tc.tile_pool(name="tiny", bufs=4) as tiny,
            tc.tile_pool(name="wpool", bufs=4) as wpool,
            tc.tile_pool(name="wopool", bufs=16) as wopool,
            tc.tile_pool(name="small", bufs=1) as small,
            tc.tile_pool(name="dram", bufs=1, space="DRAM") as dram,
        ):
            ident = const.tile([P, P], F32)
            make_identity(nc, ident)
            epsb = const.tile([P, 1], F32)
            nc.vector.memset(epsb, EPS / D)
            maskt = small.tile([P, NPT, NCORE], F32)
            nc.sync.dma_start(
                out=maskt[:],
                in_=mask_in.rearrange("p (a b) -> p a b", a=NPT))

            xnT = [xny.tile([P, CHUNK], F16, tag="xny", name=f"xnT{d}")
                   for d in range(NPT)]

            # ---- Phase A1: rmsnorm + transpose to [channel, token] ----
            # ACT does only the Sqrt; everything else lives on DVE/Pool so
            # the activation table is not thrashed.
            with tc.tile_pool(name="pst", bufs=4, space="PSUM") as pstp:
                for m in range(NM):
                    xm = scr.tile([P, D], F32, tag="x", name=f"x{m}")
                    nc.sync.dma_start(out=xm[:], in_=x_in[m * P:(m + 1) * P, :])
                    xn = scr.tile([P, D], F32, tag="xn", name=f"xn{m}")
                    ss = tiny.tile([P, 1], F32, tag="ss", name=f"ss{m}")
                    nc.vector.tensor_mul(xn[:], xm[:], xm[:])
                    nc.vector.tensor_reduce(ss[:], xn[:],
                                            axis=mybir.AxisListType.X,
                                            op=ALU.add)
                    sd = tiny.tile([P, 1], F32, tag="sd", name=f"sd{m}")
                    nc.scalar.activation(sd[:], ss[:], AFT.Sqrt,
                                         bias=epsb[:], scale=1.0 / D)
                    inv = tiny.tile([P, 1], F32, tag="inv", name=f"inv{m}")
                    nc.vector.reciprocal(inv[:], sd[:])
                    nc.scalar.activation(xn[:], xm[:], AFT.Copy, scale=inv[:])
                    for d in range(NPT):
                        pst = pstp.tile([P, P], F32, tag="pst",
                                        name=f"pst{m}_{d}")
                        nc.tensor.transpose(pst[:], xn[:, d * P:(d + 1) * P],
                                            ident[:])
                        nc.scalar.activation(xnT[d][:, m * P:(m + 1) * P],
                                             pst[:], AFT.Copy)

            a_t = [persist.tile([P, CHUNK], F32, tag=f"a{p}", name=f"a{p}")
                   for p in range(NPT)]
            kv_t = [persist.tile([P, CHUNK], F32, tag=f"kv{p}", name=f"kv{p}")
                    for p in range(NPT)]
            sl_t = [persist.tile([P, CHUNK], F32, tag=f"sl{p}", name=f"sl{p}")
                    for p in range(NPT)]
            ca_t = [persist.tile([P, CHUNK], F32, tag=f"ca{p}", name=f"ca{p}")
                    for p in range(NPT)]
            summ = small.tile([P, SW], F32)

            # ---- Phase A2a: k,v,a projections + local scans + summaries ----
            with tc.tile_pool(name="psa", bufs=6, space="PSUM") as psa:
                for p in range(NPT):
                    ks = None
                    for wname, key in (("wk", "k"), ("wv", "v"), ("wa", "a")):
                        pts = [psa.tile([P, H], F32, tag="psa",
                                        name=f"ps_{key}{p}h{h}")
                               for h in range(2)]
                        wt = wpool.tile([P, D], F16, tag="w",
                                        name=f"w_{key}{p}")
                        nc.sync.dma_start(out=wt[:], in_=w_in[wname][p])
                        for k in range(NPT):
                            for h in range(2):
                                nc.tensor.matmul(
                                    pts[h][:],
                                    lhsT=wt[:, k * P:(k + 1) * P],
                                    rhs=xnT[k][:, h * H:(h + 1) * H],
                                    start=(k == 0), stop=(k == NPT - 1))
                        for h in range(2):
                            hs = slice(h * H, (h + 1) * H)
                            if key == "k":
                                if ks is None:
                                    ks = [scr.tile([P, H], F32, tag="ks",
                                                   name=f"ks{p}h{hh}")
                                          for hh in range(2)]
                                nc.vector.tensor_copy(ks[h][:], pts[h][:])
                            elif key == "v":
                                nc.vector.tensor_mul(
                                    kv_t[p][:, hs], pts[h][:], ks[h][:])
                            elif key == "a":
                                nc.scalar.activation(a_t[p][:, hs],
                                                     pts[h][:], AFT.Sigmoid)
                    nc.vector.tensor_tensor_scan(
                        sl_t[p][:], a_t[p][:], kv_t[p][:], 0.0,
                        op0=ALU.mult, op1=ALU.add)
                    nc.vector.tensor_tensor_scan(
                        ca_t[p][:], a_t[p][:], a_t[p][:], 1.0,
                        op0=ALU.mult, op1=ALU.bypass)
                    nc.vector.tensor_copy(summ[:, p:p + 1],
                                          ca_t[p][:, CHUNK - 1:CHUNK])
                    nc.vector.tensor_copy(summ[:, NPT + p:NPT + p + 1],
                                          sl_t[p][:, CHUNK - 1:CHUNK])

            # ---- summary exchange: single AllGather, triggered ASAP ----
            import os as _os
            _nocc = bool(int(_os.environ.get("NOCC", "0")))
            cc_in = dram.tile([P, SW], F32, name="cc_in")
            cc_out = dram.tile([NCORE, P, SW], F32, addr_space="Shared",
                               name="cc_out")
            nc.sync.dma_start(out=cc_in[:], in_=summ[:])
            if not _nocc:
                nc.gpsimd.collective_compute(
                    "AllGather", ALU.bypass,
                    replica_groups=[list(range(NCORE))],
                    ins=[cc_in[:]], outs=[cc_out[:]])

            # prefetch all output-projection weights while the gather flies
            woh = [[wopool.tile([P, H], F16, tag="woh", name=f"wo{h}k{k}")
                    for k in range(NPT)] for h in range(2)]
            for h in range(2):
                for k in range(NPT):
                    nc.sync.dma_start(out=woh[h][k][:], in_=w_in["wo"][h, k])

            # ---- Phase A2b: g,q projections; P = qg*sl, C = qg*ca ----
            with tc.tile_pool(name="psb", bufs=4, space="PSUM") as psb:
                for p in range(NPT):
                    gs = [scr.tile([P, H], F32, tag="gs", name=f"gs{p}h{hh}")
                          for hh in range(2)]
                    qg_p = scr.tile([P, CHUNK], F32, tag="qg", name=f"qg{p}")
                    for wname, key in (("wg", "g"), ("wq", "q")):
                        pts = [psb.tile([P, H], F32, tag="psb",
                                        name=f"ps_{key}{p}h{h}")
                               for h in range(2)]
                        wt = wpool.tile([P, D], F16, tag="w",
                                        name=f"w_{key}{p}")
                        nc.sync.dma_start(out=wt[:], in_=w_in[wname][p])
                        for k in range(NPT):
                            for h in range(2):
                                nc.tensor.matmul(
                                    pts[h][:],
                                    lhsT=wt[:, k * P:(k + 1) * P],
                                    rhs=xnT[k][:, h * H:(h + 1) * H],
                                    start=(k == 0), stop=(k == NPT - 1))
                        for h in range(2):
                            if key == "g":
                                nc.scalar.activation(gs[h][:], pts[h][:],
                                                     AFT.Silu)
                            else:
                                nc.vector.scalar_tensor_tensor(
                                    out=qg_p[:, h * H:(h + 1) * H],
                                    in0=pts[h][:], scalar=1.0 / YSC,
                                    in1=gs[h][:],
                                    op0=ALU.mult, op1=ALU.mult)
                    # P = qg * s_local (overwrites kv); C = qg * cumA
                    # (overwrites a)
                    nc.vector.tensor_mul(kv_t[p][:], qg_p[:], sl_t[p][:])
                    nc.vector.tensor_mul(a_t[p][:], qg_p[:], ca_t[p][:])

            # ---- gather consume + prefix combine ----
            gath = small.tile([P, NCORE * SW], F32)
            sin = small.tile([P, NPT], F32)
            if _nocc:
                nc.vector.memset(gath[:], 0.0)
            else:
                for c in range(NCORE):
                    nc.gpsimd.dma_start(
                        out=gath[:, c * SW:(c + 1) * SW], in_=cc_out[c])

            def A_of(j):
                return gath[:, j * SW: j * SW + NPT]

            def s_of(j):
                return gath[:, j * SW + NPT: j * SW + SW]

            cand = small.tile([P, NPT, NCORE], F32, name="cand")
            nc.vector.memset(cand[:], 0.0)
            u = small.tile([P, NPT], F32, name="u")
            tmp = small.tile([P, NPT], F32, name="tmp")
            for g in range(GROUPS):
                base = g * CPG
                nc.vector.tensor_copy(u[:], s_of(base))
                nc.vector.tensor_copy(cand[:, :, base + 1], u[:])
                for jj in range(2, CPG):
                    nc.vector.tensor_mul(tmp[:], A_of(base + jj - 1), u[:])
                    nc.vector.tensor_add(u[:], tmp[:], s_of(base + jj - 1))
                    nc.vector.tensor_copy(cand[:, :, base + jj], u[:])
            masked = small.tile([P, NPT, NCORE], F32, name="masked")
            nc.vector.tensor_mul(masked[:], cand[:], maskt[:])
            nc.vector.tensor_reduce(sin[:], masked[:],
                                    axis=mybir.AxisListType.X, op=ALU.add)

            # ---- Phase B: apply carry, output projection ----
            y_t = [xny.tile([P, CHUNK], F16, tag="xny", name=f"y{p}")
                   for p in range(NPT)]
            for m in range(NM):
                ms = slice(m * P, (m + 1) * P)
                for p in range(NPT):
                    nc.vector.scalar_tensor_tensor(
                        out=y_t[p][:, ms], in0=a_t[p][:, ms],
                        scalar=sin[:, p:p + 1], in1=kv_t[p][:, ms],
                        op0=ALU.mult, op1=ALU.add)

            with tc.tile_pool(name="pso", bufs=4, space="PSUM") as pso:
                for h in range(2):
                    for m in range(NM):
                        po = pso.tile([P, H], F32, tag="pso",
                                      name=f"po{h}m{m}")
                        for k in range(NPT):
                            nc.tensor.matmul(
                                po[:], lhsT=y_t[k][:, m * P:(m + 1) * P],
                                rhs=woh[h][k][:],
                                start=(k == 0), stop=(k == NPT - 1))
                        ostg = scr.tile([P, H], F32, tag="ostg",
                                        name=f"ostg{h}m{m}", bufs=2)
                        nc.scalar.activation(ostg[:], po[:], AFT.Copy)
                        nc.sync.dma_start(
                            out=out_t[m * P:(m + 1) * P, h * H:(h + 1) * H],
                            in_=ostg[:])

    nc.compile()
    return nc


def tc_ctx(nc):
    return tile.TileContext(nc)


def _get_nc():
    if "nc" not in _CACHE:
        _CACHE["nc"] = _build()
    return _CACHE["nc"]


def _blk_proj(w):
    # [din, dout] -> [p, r, (k, c)]: per dout-ptile slab, contiguous
    return np.ascontiguousarray(
        w.reshape(NPT, P, NPT, P).transpose(2, 1, 0, 3).reshape(NPT, P, D)
        .astype(np.float16))


def _blk_out(w):
    # [din, dout] -> [h, k, r, c]
    return np.ascontiguousarray(
        w.reshape(NPT, P, 2, H).transpose(2, 0, 1, 3).astype(np.float16))


def _make_in_maps(x, gamma, wq, wk, wv, wa, wg, wo):
    w_eff = {
        "wq": _blk_proj(gamma[:, None] * wq),
        "wk": _blk_proj(gamma[:, None] * wk),
        "wv": _blk_proj(gamma[:, None] * wv),
        "wa": _blk_proj(gamma[:, None] * wa),
        "wg": _blk_proj(gamma[:, None] * wg),
        "wo": _blk_out(wo * YSC),
    }
    in_maps = []
    for c in range(NCORE):
        b, ch = divmod(c, CPG)
        mask = np.zeros((P, NPT, NCORE), dtype=np.float32)
        mask[:, :, c] = 1.0
        in_maps.append({
            "x": np.ascontiguousarray(
                x[b, ch * CHUNK:(ch + 1) * CHUNK, :], dtype=np.float32),
            "mask": mask.reshape(P, NPT * NCORE),
            **w_eff,
        })
    return in_maps


def run_device(in_maps, trace=False, **kw):
    return run_bass_kernel_spmd(_get_nc(), in_maps, list(range(NCORE)),
                                trace=trace, **kw)


def _assemble(results):
    out = np.empty((B, S, D), dtype=np.float32)
    for c in range(NCORE):
        b, ch = divmod(c, CPG)
        out[b, ch * CHUNK:(ch + 1) * CHUNK, :] = results[c]["out"]
    return out


def kernel(x, gamma, wq, wk, wv, wa, wg, wo):
    in_maps = _make_in_maps(np.asarray(x), np.asarray(gamma), np.asarray(wq),
                            np.asarray(wk), np.asarray(wv), np.asarray(wa),
                            np.asarray(wg), np.asarray(wo))
    res = run_device(in_maps)
    return _assemble(res.results)


# revision 12
# speedup vs baseline: 1.2177x; 1.2177x over previous
"""GateLoop fused Bass/Tile kernel for Trainium2, SPMD over 8 NeuronCores.

Problem (B=2, S=4096, D=1024):
    xn = rmsnorm(x) * gamma * sqrt(D)         (sum-of-squares norm)
    q,k,v = xn@wq, xn@wk, xn@wv ; a = sigmoid(xn@wa) ; g = xn@wg
    s_t = a_t * s_{t-1} + (k_t*v_t)           (elementwise linear recurrence)
    out = (q*s * silu(g)) @ wo
a
Sharding: sequence-parallel. Core c handles batch c//4, tokens
[(c%4)*1024, (c%4+1)*1024). The cross-chunk scan carry is resolved with the
decomposition  s = s_local + cumA * s_in:  each core computes per-chunk
summaries (A_total, s_last), AllGathers them (8KB), combines prefixes
locally, and applies its incoming state as a per-channel scalar.

Schedule: the AllGather's completion is gated by the slowest core's arrival
(launch skew across the 8 PJRT devices is ~90us), so the kernel is ordered
to bank carry-independent work behind the collective: (k,v,a) projections +
local scans first -> single AllGather of both summary halves -> (g,q)
projections + wo prefetch while the gather is in flight -> only the carry
apply + output projection remain on the dependent tail.

gamma is folded into the five input-side projection weights on the host.
Matmuls run in fp16 at full (double-pumped) PE rate.
"""

import numpy as np

import concourse.bacc as bacc
import concourse.tile as tile
from concourse import mybir
from concourse.bass_utils import run_bass_kernel_spmd
from concourse.masks import make_identity

AFT = mybir.ActivationFunctionType
ALU = mybir.AluOpType
F32 = mybir.dt.float32
F16 = mybir.dt.float16

B, S, D = 2, 4096, 1024
NCORE = 8
GROUPS = 2              # batch groups of 4 cores
CPG = NCORE // GROUPS   # chunks (cores) per group
CHUNK = (B * S) // NCORE  # 1024 tokens per core
P = 128
NPT = D // P            # 8 channel ptiles
NM = CHUNK // P         # 8 token tiles
H = 512                 # psum half width (fp32 bank)
EPS = 1e-5
YSC = 4096.0            # fp16 range guard: y is carried as y/YSC
SW = 2 * NPT            # summary width: [A_total | s_last] per ptile

_CACHE = {}


def _build():
    nc = bacc.Bacc("TRN2", target_bir_lowering=False, debug=False,
                   num_devices=NCORE)
    x_in = nc.dram_tensor("x", [CHUNK, D], F32, kind="ExternalInput")
    w_in = {
        n: nc.dram_tensor(n, [NPT, P, D], F16, kind="ExternalInput")
        for n in ("wg", "wq", "wk", "wv", "wa")
    }
    w_in["wo"] = nc.dram_tensor("wo", [2, NPT, P, H], F16,
                                kind="ExternalInput")
    mask_in = nc.dram_tensor("mask", [P, NPT * NCORE], F32,
                             kind="ExternalInput")
    out_t = nc.dram_tensor("out", [CHUNK, D], F32, kind="ExternalOutput")

    with tc_ctx(nc) as tc:
        with (
            tc.tile_pool(name="const", bufs=1) as const,
            tc.tile_pool(name="xny", bufs=NPT) as xny,
            tc.tile_pool(name="persist", bufs=1) as persist,
            tc.tile_pool(name="scr", bufs=2) as scr,
            tc.tile_pool(name="tiny", bufs=4) as tiny,
            tc.tile_pool(name="wpool", bufs=4) as wpool,
            tc.tile_pool(name="wopool", bufs=16) as wopool,
            tc.tile_pool(name="small", bufs=1) as small,
            tc.tile_pool(name="dram", bufs=1, space="DRAM") as dram,
        ):
            ident = const.tile([P, P], F32)
            make_identity(nc, ident)
            epsb = const.tile([P, 1], F32)
            nc.vector.memset(epsb, EPS / D)
            maskt = small.tile([P, NPT, NCORE], F32)
            nc.sync.dma_start(
                out=maskt[:],
                in_=mask_in.rearrange("p (a b) -> p a b", a=NPT))

            xnT = [xny.tile([P, CHUNK], F16, tag="xny", name=f"xnT{d}")
                   for d in range(NPT)]

            # ---- Phase A1: rmsnorm + transpose to [channel, token] ----
            # ACT does only the Sqrt; everything else lives on DVE/Pool so
            # the activation table is not thrashed.
            with tc.tile_pool(name="pst", bufs=4, space="PSUM") as pstp:
                for m in range(NM):
                    xm = scr.tile([P, D], F32, tag="x", name=f"x{m}")
                    nc.sync.dma_start(out=xm[:], in_=x_in[m * P:(m + 1) * P, :])
                    xn = scr.tile([P, D], F32, tag="xn", name=f"xn{m}")
                    ss = tiny.tile([P, 1], F32, tag="ss", name=f"ss{m}")
                    nc.vector.tensor_mul(xn[:], xm[:], xm[:])
                    nc.vector.tensor_reduce(ss[:], xn[:],
                                            axis=mybir.AxisListType.X,
                                            op=ALU.add)
                    sd = tiny.tile([P, 1], F32, tag="sd", name=f"sd{m}")
                    nc.scalar.activation(sd[:], ss[:], AFT.Sqrt,
                                         bias=epsb[:], scale=1.0 / D)
                    inv = tiny.tile([P, 1], F32, tag="inv", name=f"inv{m}")
                    nc.vector.reciprocal(inv[:], sd[:])
                    nc.scalar.activation(xn[:], xm[:], AFT.Copy, scale=inv[:])
                    for d in range(NPT):
                        pst = pstp.tile([P, P], F32, tag="pst",
                                        name=f"pst{m}_{d}")
                        nc.tensor.transpose(pst[:], xn[:, d * P:(d + 1) * P],
                                            ident[:])
                        nc.scalar.activation(xnT[d][:, m * P:(m + 1) * P],
                                             pst[:], AFT.Copy)

            a_t = [persist.tile([P, CHUNK], F32, tag=f"a{p}", name=f"a{p}")
                   for p in range(NPT)]
            kv_t = [persist.tile([P, CHUNK], F32, tag=f"kv{p}", name=f"kv{p}")
                    for p in range(NPT)]
            sl_t = [persist.tile([P, CHUNK], F32, tag=f"sl{p}", name=f"sl{p}")
                    for p in range(NPT)]
            ca_t = [persist.tile([P, CHUNK], F32, tag=f"ca{p}", name=f"ca{p}")
                    for p in range(NPT)]
            summ = small.tile([P, SW], F32)

            # ---- Phase A2a: k,v,a projections + local scans + summaries ----
            with tc.tile_pool(name="psa", bufs=6, space="PSUM") as psa:
                for p in range(NPT):
                    ks = None
                    for wname, key in (("wk", "k"), ("wv", "v"), ("wa", "a")):
                        pts = [psa.tile([P, H], F32, tag="psa",
                                        name=f"ps_{key}{p}h{h}")
                               for h in range(2)]
                        wt = wpool.tile([P, D], F16, tag="w",
                                        name=f"w_{key}{p}")
                        nc.sync.dma_start(out=wt[:], in_=w_in[wname][p])
                        for k in range(NPT):
                            for h in range(2):
                                nc.tensor.matmul(
                                    pts[h][:],
                                    lhsT=wt[:, k * P:(k + 1) * P],
                                    rhs=xnT[k][:, h * H:(h + 1) * H],
                                    start=(k == 0), stop=(k == NPT - 1))
                        for h in range(2):
                            hs = slice(h * H, (h + 1) * H)
                            if key == "k":
                                if ks is None:
                                    ks = [scr.tile([P, H], F32, tag="ks",
                                                   name=f"ks{p}h{hh}")
                                          for hh in range(2)]
                                nc.vector.tensor_copy(ks[h][:], pts[h][:])
                            elif key == "v":
                                nc.vector.tensor_mul(
                                    kv_t[p][:, hs], pts[h][:], ks[h][:])
                            elif key == "a":
                                nc.scalar.activation(a_t[p][:, hs],
                                                     pts[h][:], AFT.Sigmoid)
                    nc.vector.tensor_tensor_scan(
                        sl_t[p][:], a_t[p][:], kv_t[p][:], 0.0,
                        op0=ALU.mult, op1=ALU.add)
                    nc.vector.tensor_tensor_scan(
                        ca_t[p][:], a_t[p][:], a_t[p][:], 1.0,
                        op0=ALU.mult, op1=ALU.bypass)
                    nc.vector.tensor_copy(summ[:, p:p + 1],
                                          ca_t[p][:, CHUNK - 1:CHUNK])
                    nc.vector.tensor_copy(summ[:, NPT + p:NPT + p + 1],
                                          sl_t[p][:, CHUNK - 1:CHUNK])

            # ---- summary exchange: single AllGather, triggered ASAP ----
            import os as _os
            _nocc = bool(int(_os.environ.get("NOCC", "0")))
            cc_in = dram.tile([P, SW], F32, name="cc_in")
            cc_out = dram.tile([NCORE, P, SW], F32, addr_space="Shared",
                               name="cc_out")
            nc.sync.dma_start(out=cc_in[:], in_=summ[:])
            if not _nocc:
                nc.gpsimd.collective_compute(
                    "AllGather", ALU.bypass,
                    replica_groups=[list(range(NCORE))],
                    ins=[cc_in[:]], outs=[cc_out[:]])

            # prefetch all output-projection weights while the gather flies
            woh = [[wopool.tile([P, H], F16, tag="woh", name=f"wo{h}k{k}")
                    for k in range(NPT)] for h in range(2)]
            for h in range(2):
                for k in range(NPT):
                    nc.sync.dma_start(out=woh[h][k][:], in_=w_in["wo"][h, k])

            # ---- Phase A2b: g,q projections; P = qg*sl, C = qg*ca ----
            with tc.tile_pool(name="psb", bufs=4, space="PSUM") as psb:
                for p in range(NPT):
                    gs = [scr.tile([P, H], F32, tag="gs", name=f"gs{p}h{hh}")
                          for hh in range(2)]
                    qg_p = scr.tile([P, CHUNK], F32, tag="qg", name=f"qg{p}")
                    for wname, key in (("wg", "g"), ("wq", "q")):
                        pts = [psb.tile([P, H], F32, tag="psb",
                                        name=f"ps_{key}{p}h{h}")
                               for h in range(2)]
                        wt = wpool.tile([P, D], F16, tag="w",
                                        name=f"w_{key}{p}")
                        nc.sync.dma_start(out=wt[:], in_=w_in[wname][p])
                        for k in range(NPT):
                            for h in range(2):
                                nc.tensor.matmul(
                                    pts[h][:],
                                    lhsT=wt[:, k * P:(k + 1) * P],
                                    rhs=xnT[k][:, h * H:(h + 1) * H],
                                    start=(k == 0), stop=(k == NPT - 1))
                        for h in range(2):
                            if key == "g":
                                nc.scalar.activation(gs[h][:], pts[h][:],
                                                     AFT.Silu)
                            else:
                                nc.vector.scalar_tensor_tensor(
                                    out=qg_p[:, h * H:(h + 1) * H],
                                    in0=pts[h][:], scalar=1.0 / YSC,
                                    in1=gs[h][:],
                                    op0=ALU.mult, op1=ALU.mult)
                    # P = qg * s_local (overwrites kv); C = qg * cumA
                    # (overwrites a)
                    nc.vector.tensor_mul(kv_t[p][:], qg_p[:], sl_t[p][:])
                    nc.vector.tensor_mul(a_t[p][:], qg_p[:], ca_t[p][:])

            # ---- gather consume + prefix combine ----
            gath = small.tile([P, NCORE * SW], F32)
            sin = small.tile([P, NPT], F32)
            if _nocc:
                nc.vector.memset(gath[:], 0.0)
            else:
                for c in range(NCORE):
                    nc.gpsimd.dma_start(
                        out=gath[:, c * SW:(c + 1) * SW], in_=cc_out[c])

            def A_of(j):
                return gath[:, j * SW: j * SW + NPT]

            def s_of(j):
                return gath[:, j * SW + NPT: j * SW + SW]

            cand = small.tile([P, NPT, NCORE], F32, name="cand")
            u = small.tile([P, NPT], F32, name="u")
            tmp = small.tile([P, NPT], F32, name="tmp")
            masked = small.tile([P, NPT, NCORE], F32, name="masked")
            y_t = [xny.tile([P, CHUNK], F16, tag="xny", name=f"y{p}")
                   for p in range(NPT)]
            # Schedule the gather-gated combine/apply at the tail of the
            # vector queue: the Tile scheduler orders each engine's FIFO by
            # estimated ready time, and without the override it hoists these
            # (collective-gated) ops ahead of the remaining A2b evictions,
            # head-blocking the DVE queue behind the slowest core's arrival.
            with tc.tile_wait_until(1.0):
                nc.vector.memset(cand[:], 0.0)
                for g in range(GROUPS):
                    base = g * CPG
                    nc.vector.tensor_copy(u[:], s_of(base))
                    nc.vector.tensor_copy(cand[:, :, base + 1], u[:])
                    for jj in range(2, CPG):
                        nc.vector.tensor_mul(tmp[:], A_of(base + jj - 1),
                                             u[:])
                        nc.vector.tensor_add(u[:], tmp[:],
                                             s_of(base + jj - 1))
                        nc.vector.tensor_copy(cand[:, :, base + jj], u[:])
                nc.vector.tensor_mul(masked[:], cand[:], maskt[:])
                nc.vector.tensor_reduce(sin[:], masked[:],
                                        axis=mybir.AxisListType.X,
                                        op=ALU.add)

                # ---- Phase B: apply carry, output projection ----
                for m in range(NM):
                    ms = slice(m * P, (m + 1) * P)
                    for p in range(NPT):
                        nc.vector.scalar_tensor_tensor(
                            out=y_t[p][:, ms], in0=a_t[p][:, ms],
                            scalar=sin[:, p:p + 1], in1=kv_t[p][:, ms],
                            op0=ALU.mult, op1=ALU.add)

            with tc.tile_pool(name="pso", bufs=4, space="PSUM") as pso:
                for h in range(2):
                    for m in range(NM):
                        po = pso.tile([P, H], F32, tag="pso",
                                      name=f"po{h}m{m}")
                        for k in range(NPT):
                            nc.tensor.matmul(
                                po[:], lhsT=y_t[k][:, m * P:(m + 1) * P],
                                rhs=woh[h][k][:],
                                start=(k == 0), stop=(k == NPT - 1))
                        ostg = scr.tile([P, H], F32, tag="ostg",
                                        name=f"ostg{h}m{m}", bufs=2)
                        nc.scalar.activation(ostg[:], po[:], AFT.Copy)
                        nc.sync.dma_start(
                            out=out_t[m * P:(m + 1) * P, h * H:(h + 1) * H],
                            in_=ostg[:])

    nc.compile()
    return nc


def tc_ctx(nc):
    return tile.TileContext(nc)


def _get_nc():
    if "nc" not in _CACHE:
        _CACHE["nc"] = _build()
    return _CACHE["nc"]


def _blk_proj(w):
    # [din, dout] -> [p, r, (k, c)]: per dout-ptile slab, contiguous
    return np.ascontiguousarray(
        w.reshape(NPT, P, NPT, P).transpose(2, 1, 0, 3).reshape(NPT, P, D)
        .astype(np.float16))


def _blk_out(w):
    # [din, dout] -> [h, k, r, c]
    return np.ascontiguousarray(
        w.reshape(NPT, P, 2, H).transpose(2, 0, 1, 3).astype(np.float16))


def _make_in_maps(x, gamma, wq, wk, wv, wa, wg, wo):
    w_eff = {
        "wq": _blk_proj(gamma[:, None] * wq),
        "wk": _blk_proj(gamma[:, None] * wk),
        "wv": _blk_proj(gamma[:, None] * wv),
        "wa": _blk_proj(gamma[:, None] * wa),
        "wg": _blk_proj(gamma[:, None] * wg),
        "wo": _blk_out(wo * YSC),
    }
    in_maps = []
    for c in range(NCORE):
        b, ch = divmod(c, CPG)
        mask = np.zeros((P, NPT, NCORE), dtype=np.float32)
        mask[:, :, c] = 1.0
        in_maps.append({
            "x": np.ascontiguousarray(
                x[b, ch * CHUNK:(ch + 1) * CHUNK, :], dtype=np.float32),
            "mask": mask.reshape(P, NPT * NCORE),
            **w_eff,
        })
    return in_maps


def run_device(in_maps, trace=False, **kw):
    return run_bass_kernel_spmd(_get_nc(), in_maps, list(range(NCORE)),
                                trace=trace, **kw)


def _assemble(results):
    out = np.empty((B, S, D), dtype=np.float32)
    for c in range(NCORE):
        b, ch = divmod(c, CPG)
        out[b, ch * CHUNK:(ch + 1) * CHUNK, :] = results[c]["out"]
    return out


def kernel(x, gamma, wq, wk, wv, wa, wg, wo):
    in_maps = _make_in_maps(np.asarray(x), np.asarray(gamma), np.asarray(wq),
                            np.asarray(wk), np.asarray(wv), np.asarray(wa),
                            np.asarray(wg), np.asarray(wo))
    res = run_device(in_maps)
    return _assemble(res.results)


# revision 13
# speedup vs baseline: 1.2310x; 1.0109x over previous
"""GateLoop fused Bass/Tile kernel for Trainium2, SPMD over 8 NeuronCores.

Problem (B=2, S=4096, D=1024):
    xn = rmsnorm(x) * gamma * sqrt(D)         (sum-of-squares norm)
    q,k,v = xn@wq, xn@wk, xn@wv ; a = sigmoid(xn@wa) ; g = xn@wg
    s_t = a_t * s_{t-1} + (k_t*v_t)           (elementwise linear recurrence)
    out = (q*s * silu(g)) @ wo
a
Sharding: sequence-parallel. Core c handles batch c//4, tokens
[(c%4)*1024, (c%4+1)*1024). The cross-chunk scan carry is resolved with the
decomposition  s = s_local + cumA * s_in:  each core computes per-chunk
summaries (A_total, s_last), AllGathers them (8KB), combines prefixes
locally, and applies its incoming state as a per-channel scalar.

Schedule: the AllGather's completion is gated by the slowest core's arrival
(launch skew across the 8 PJRT devices is ~90us), so the kernel is ordered
to bank carry-independent work behind the collective: (k,v,a) projections +
local scans first -> single AllGather of both summary halves -> (g,q)
projections + wo prefetch while the gather is in flight -> only the carry
apply + output projection remain on the dependent tail.

gamma is folded into the five input-side projection weights on the host.
Matmuls run in fp16 at full (double-pumped) PE rate.
"""

import numpy as np

import concourse.bacc as bacc
import concourse.tile as tile
from concourse import mybir
from concourse.bass_utils import run_bass_kernel_spmd
from concourse.masks import make_identity

AFT = mybir.ActivationFunctionType
ALU = mybir.AluOpType
F32 = mybir.dt.float32
F16 = mybir.dt.float16

B, S, D = 2, 4096, 1024
NCORE = 8
GROUPS = 2              # batch groups of 4 cores
CPG = NCORE // GROUPS   # chunks (cores) per group
CHUNK = (B * S) // NCORE  # 1024 tokens per core
P = 128
NPT = D // P            # 8 channel ptiles
NM = CHUNK // P         # 8 token tiles
H = 512                 # psum half width (fp32 bank)
EPS = 1e-5
YSC = 4096.0            # fp16 range guard: y is carried as y/YSC
SW = 2 * NPT            # summary width: [A_total | s_last] per ptile

_CACHE = {}


def _build():
    nc = bacc.Bacc("TRN2", target_bir_lowering=False, debug=False,
                   num_devices=NCORE)
    x_in = nc.dram_tensor("x", [CHUNK, D], F32, kind="ExternalInput")
    w_in = {
        n: nc.dram_tensor(n, [NPT, P, D], F16, kind="ExternalInput")
        for n in ("wg", "wq", "wk", "wv", "wa")
    }
    w_in["wo"] = nc.dram_tensor("wo", [2, NPT, P, H], F16,
                                kind="ExternalInput")
    mask_in = nc.dram_tensor("mask", [P, NPT * NCORE], F32,
                             kind="ExternalInput")
    out_t = nc.dram_tensor("out", [CHUNK, D], F32, kind="ExternalOutput")

    with tc_ctx(nc) as tc:
        with (
            tc.tile_pool(name="const", bufs=1) as const,
            tc.tile_pool(name="xny", bufs=NPT) as xny,
            tc.tile_pool(name="persist", bufs=1) as persist,
            tc.tile_pool(name="scr", bufs=2) as scr,
            tc.tile_pool(name="tiny", bufs=4) as tiny,
            tc.tile_pool(name="wpool", bufs=4) as wpool,
            tc.tile_pool(name="wopool", bufs=16) as wopool,
            tc.tile_pool(name="small", bufs=1) as small,
            tc.tile_pool(name="dram", bufs=1, space="DRAM") as dram,
        ):
            ident = const.tile([P, P], F32)
            make_identity(nc, ident)
            epsb = const.tile([P, 1], F32)
            nc.vector.memset(epsb, EPS / D)
            warm = const.tile([P, 1], F32)
            nc.scalar.activation(warm[:], epsb[:], AFT.Sqrt)
            maskt = small.tile([P, NPT, NCORE], F32)
            nc.sync.dma_start(
                out=maskt[:],
                in_=mask_in.rearrange("p (a b) -> p a b", a=NPT))

            xnT = [[persist.tile([P, H], F16, tag=f"xT{h}{d}",
                                 name=f"xnT{h}_{d}")
                    for d in range(NPT)] for h in range(2)]

            # ---- Phase A1: rmsnorm + transpose to [channel, token] ----
            # ACT does only the Sqrt; everything else lives on DVE/Pool so
            # the activation table is not thrashed.
            with tc.tile_pool(name="pst", bufs=4, space="PSUM") as pstp:
                for m in range(NM):
                    xm = scr.tile([P, D], F32, tag="x", name=f"x{m}")
                    nc.sync.dma_start(out=xm[:], in_=x_in[m * P:(m + 1) * P, :])
                    xn = scr.tile([P, D], F32, tag="xn", name=f"xn{m}")
                    ss = tiny.tile([P, 1], F32, tag="ss", name=f"ss{m}")
                    nc.vector.tensor_mul(xn[:], xm[:], xm[:])
                    nc.vector.tensor_reduce(ss[:], xn[:],
                                            axis=mybir.AxisListType.X,
                                            op=ALU.add)
                    sd = tiny.tile([P, 1], F32, tag="sd", name=f"sd{m}")
                    nc.scalar.activation(sd[:], ss[:], AFT.Sqrt,
                                         bias=epsb[:], scale=1.0 / D)
                    inv = tiny.tile([P, 1], F32, tag="inv", name=f"inv{m}")
                    nc.vector.reciprocal(inv[:], sd[:])
                    nc.scalar.activation(xn[:], xm[:], AFT.Copy, scale=inv[:])
                    mh, mq = divmod(m, NM // 2)
                    for d in range(NPT):
                        pst = pstp.tile([P, P], F32, tag="pst",
                                        name=f"pst{m}_{d}")
                        nc.tensor.transpose(pst[:], xn[:, d * P:(d + 1) * P],
                                            ident[:])
                        nc.scalar.activation(
                            xnT[mh][d][:, mq * P:(mq + 1) * P],
                            pst[:], AFT.Copy)

            a_t = [persist.tile([P, CHUNK], F32, tag=f"a{p}", name=f"a{p}")
                   for p in range(NPT)]
            kv_t = [persist.tile([P, CHUNK], F32, tag=f"kv{p}", name=f"kv{p}")
                    for p in range(NPT)]
            sl_t = [persist.tile([P, CHUNK], F32, tag=f"sl{p}", name=f"sl{p}")
                    for p in range(NPT)]
            ca_t = [persist.tile([P, CHUNK], F32, tag=f"ca{p}", name=f"ca{p}")
                    for p in range(NPT)]
            summ = small.tile([P, SW], F32)

            # ---- Phase A2a: k,v,a projections + local scans + summaries ----
            with tc.tile_pool(name="psa", bufs=6, space="PSUM") as psa:
                for p in range(NPT):
                    ks = None
                    for wname, key in (("wk", "k"), ("wv", "v"), ("wa", "a")):
                        pts = [psa.tile([P, H], F32, tag="psa",
                                        name=f"ps_{key}{p}h{h}")
                               for h in range(2)]
                        wt = wpool.tile([P, D], F16, tag="w",
                                        name=f"w_{key}{p}")
                        nc.sync.dma_start(out=wt[:], in_=w_in[wname][p])
                        for h in range(2):
                            for k in range(NPT):
                                nc.tensor.matmul(
                                    pts[h][:],
                                    lhsT=wt[:, k * P:(k + 1) * P],
                                    rhs=xnT[h][k][:],
                                    start=(k == 0), stop=(k == NPT - 1))
                        for h in range(2):
                            hs = slice(h * H, (h + 1) * H)
                            if key == "k":
                                if ks is None:
                                    ks = [scr.tile([P, H], F32, tag="ks",
                                                   name=f"ks{p}h{hh}")
                                          for hh in range(2)]
                                nc.vector.tensor_copy(ks[h][:], pts[h][:])
                            elif key == "v":
                                nc.vector.tensor_mul(
                                    kv_t[p][:, hs], pts[h][:], ks[h][:])
                            elif key == "a":
                                nc.scalar.activation(a_t[p][:, hs],
                                                     pts[h][:], AFT.Sigmoid)
                    nc.vector.tensor_tensor_scan(
                        sl_t[p][:], a_t[p][:], kv_t[p][:], 0.0,
                        op0=ALU.mult, op1=ALU.add)
                    # A_total = prod(a) via mult-reduce; the cumulative-A
                    # scan itself is deferred into A2b where it overlaps
                    # the g/q matmuls instead of delaying the gather.
                    nc.vector.tensor_reduce(summ[:, p:p + 1], a_t[p][:],
                                            axis=mybir.AxisListType.X,
                                            op=ALU.mult)
                    nc.vector.tensor_copy(summ[:, NPT + p:NPT + p + 1],
                                          sl_t[p][:, CHUNK - 1:CHUNK])

            # ---- summary exchange: single AllGather, triggered ASAP ----
            import os as _os
            _nocc = bool(int(_os.environ.get("NOCC", "0")))
            cc_in = dram.tile([P, SW], F32, name="cc_in")
            cc_out = dram.tile([NCORE, P, SW], F32, addr_space="Shared",
                               name="cc_out")
            nc.gpsimd.dma_start(out=cc_in[:], in_=summ[:])
            if not _nocc:
                nc.gpsimd.collective_compute(
                    "AllGather", ALU.bypass,
                    replica_groups=[list(range(NCORE))],
                    ins=[cc_in[:]], outs=[cc_out[:]])

            # prefetch all output-projection weights while the gather flies
            woh = [[wopool.tile([P, H], F16, tag="woh", name=f"wo{h}k{k}")
                    for k in range(NPT)] for h in range(2)]
            for h in range(2):
                for k in range(NPT):
                    nc.sync.dma_start(out=woh[h][k][:], in_=w_in["wo"][h, k])

            # ---- Phase A2b: g,q projections; P = qg*sl, C = qg*ca ----
            with tc.tile_pool(name="psb", bufs=4, space="PSUM") as psb:
                for p in range(NPT):
                    gs = [scr.tile([P, H], F32, tag="gs", name=f"gs{p}h{hh}")
                          for hh in range(2)]
                    qg_p = scr.tile([P, CHUNK], F32, tag="qg", name=f"qg{p}")
                    for wname, key in (("wg", "g"), ("wq", "q")):
                        pts = [psb.tile([P, H], F32, tag="psb",
                                        name=f"ps_{key}{p}h{h}")
                               for h in range(2)]
                        wt = wpool.tile([P, D], F16, tag="w",
                                        name=f"w_{key}{p}")
                        nc.sync.dma_start(out=wt[:], in_=w_in[wname][p])
                        for h in range(2):
                            for k in range(NPT):
                                nc.tensor.matmul(
                                    pts[h][:],
                                    lhsT=wt[:, k * P:(k + 1) * P],
                                    rhs=xnT[h][k][:],
                                    start=(k == 0), stop=(k == NPT - 1))
                        for h in range(2):
                            if key == "g":
                                nc.scalar.activation(gs[h][:], pts[h][:],
                                                     AFT.Silu)
                            else:
                                nc.vector.scalar_tensor_tensor(
                                    out=qg_p[:, h * H:(h + 1) * H],
                                    in0=pts[h][:], scalar=1.0 / YSC,
                                    in1=gs[h][:],
                                    op0=ALU.mult, op1=ALU.mult)
                    nc.vector.tensor_tensor_scan(
                        ca_t[p][:], a_t[p][:], a_t[p][:], 1.0,
                        op0=ALU.mult, op1=ALU.bypass)
                    # P = qg * s_local (overwrites kv); C = qg * cumA
                    # (overwrites a)
                    nc.vector.tensor_mul(kv_t[p][:], qg_p[:], sl_t[p][:])
                    nc.vector.tensor_mul(a_t[p][:], qg_p[:], ca_t[p][:])

            # ---- gather consume + prefix combine ----
            gath = small.tile([P, NCORE * SW], F32)
            sin = small.tile([P, NPT], F32)
            if _nocc:
                nc.vector.memset(gath[:], 0.0)
            else:
                for c in range(NCORE):
                    nc.gpsimd.dma_start(
                        out=gath[:, c * SW:(c + 1) * SW], in_=cc_out[c])

            def A_of(j):
                return gath[:, j * SW: j * SW + NPT]

            def s_of(j):
                return gath[:, j * SW + NPT: j * SW + SW]

            cand = small.tile([P, NPT, NCORE], F32, name="cand")
            u = small.tile([P, NPT], F32, name="u")
            tmp = small.tile([P, NPT], F32, name="tmp")
            masked = small.tile([P, NPT, NCORE], F32, name="masked")
            y_t = [persist.tile([P, CHUNK], F16, tag=f"sl{p}",
                                name=f"y{p}") for p in range(NPT)]
            # Schedule the gather-gated combine/apply at the tail of the
            # vector queue: the Tile scheduler orders each engine's FIFO by
            # estimated ready time, and without the override it hoists these
            # (collective-gated) ops ahead of the remaining A2b evictions,
            # head-blocking the DVE queue behind the slowest core's arrival.
            with tc.tile_wait_until(1.0):
                nc.vector.memset(cand[:], 0.0)
                for g in range(GROUPS):
                    base = g * CPG
                    nc.vector.tensor_copy(u[:], s_of(base))
                    nc.vector.tensor_copy(cand[:, :, base + 1], u[:])
                    for jj in range(2, CPG):
                        nc.vector.tensor_mul(tmp[:], A_of(base + jj - 1),
                                             u[:])
                        nc.vector.tensor_add(u[:], tmp[:],
                                             s_of(base + jj - 1))
                        nc.vector.tensor_copy(cand[:, :, base + jj], u[:])
                nc.vector.tensor_mul(masked[:], cand[:], maskt[:])
                nc.vector.tensor_reduce(sin[:], masked[:],
                                        axis=mybir.AxisListType.X,
                                        op=ALU.add)

                # ---- Phase B: apply carry, output projection ----
                for m in range(NM):
                    ms = slice(m * P, (m + 1) * P)
                    for p in range(NPT):
                        nc.vector.scalar_tensor_tensor(
                            out=y_t[p][:, ms], in0=a_t[p][:, ms],
                            scalar=sin[:, p:p + 1], in1=kv_t[p][:, ms],
                            op0=ALU.mult, op1=ALU.add)

            with tc.tile_pool(name="pso", bufs=4, space="PSUM") as pso:
                for h in range(2):
                    for m in range(NM):
                        po = pso.tile([P, H], F32, tag="pso",
                                      name=f"po{h}m{m}")
                        for k in range(NPT):
                            nc.tensor.matmul(
                                po[:], lhsT=y_t[k][:, m * P:(m + 1) * P],
                                rhs=woh[h][k][:],
                                start=(k == 0), stop=(k == NPT - 1))
                        ostg = scr.tile([P, H], F32, tag="ostg",
                                        name=f"ostg{h}m{m}", bufs=2)
                        nc.scalar.activation(ostg[:], po[:], AFT.Copy)
                        nc.sync.dma_start(
                            out=out_t[m * P:(m + 1) * P, h * H:(h + 1) * H],
                            in_=ostg[:])

    nc.compile()
    return nc


def tc_ctx(nc):
    return tile.TileContext(nc)


def _get_nc():
    if "nc" not in _CACHE:
        _CACHE["nc"] = _build()
    return _CACHE["nc"]


def _blk_proj(w):
    # [din, dout] -> [p, r, (k, c)]: per dout-ptile slab, contiguous
    return np.ascontiguousarray(
        w.reshape(NPT, P, NPT, P).transpose(2, 1, 0, 3).reshape(NPT, P, D)
        .astype(np.float16))


def _blk_out(w):
    # [din, dout] -> [h, k, r, c]
    return np.ascontiguousarray(
        w.reshape(NPT, P, 2, H).transpose(2, 0, 1, 3).astype(np.float16))


def _make_in_maps(x, gamma, wq, wk, wv, wa, wg, wo):
    w_eff = {
        "wq": _blk_proj(gamma[:, None] * wq),
        "wk": _blk_proj(gamma[:, None] * wk),
        "wv": _blk_proj(gamma[:, None] * wv),
        "wa": _blk_proj(gamma[:, None] * wa),
        "wg": _blk_proj(gamma[:, None] * wg),
        "wo": _blk_out(wo * YSC),
    }
    in_maps = []
    for c in range(NCORE):
        b, ch = divmod(c, CPG)
        mask = np.zeros((P, NPT, NCORE), dtype=np.float32)
        mask[:, :, c] = 1.0
        in_maps.append({
            "x": np.ascontiguousarray(
                x[b, ch * CHUNK:(ch + 1) * CHUNK, :], dtype=np.float32),
            "mask": mask.reshape(P, NPT * NCORE),
            **w_eff,
        })
    return in_maps


def run_device(in_maps, trace=False, **kw):
    return run_bass_kernel_spmd(_get_nc(), in_maps, list(range(NCORE)),
                                trace=trace, **kw)


def _assemble(results):
    out = np.empty((B, S, D), dtype=np.float32)
    for c in range(NCORE):
        b, ch = divmod(c, CPG)
        out[b, ch * CHUNK:(ch + 1) * CHUNK, :] = results[c]["out"]
    return out


def kernel(x, gamma, wq, wk, wv, wa, wg, wo):
    in_maps = _make_in_maps(np.asarray(x), np.asarray(gamma), np.asarray(wq),
                            np.asarray(wk), np.asarray(wv), np.asarray(wa),
                            np.asarray(wg), np.asarray(wo))
    res = run_device(in_maps)
    return _assemble(res.results)


# revision 18
# speedup vs baseline: 1.2432x; 1.0100x over previous
"""GateLoop fused Bass/Tile kernel for Trainium2, SPMD over 8 NeuronCores.

Problem (B=2, S=4096, D=1024):
    xn = rmsnorm(x) * gamma * sqrt(D)         (sum-of-squares norm)
    q,k,v = xn@wq, xn@wk, xn@wv ; a = sigmoid(xn@wa) ; g = xn@wg
    s_t = a_t * s_{t-1} + (k_t*v_t)           (elementwise linear recurrence)
    out = (q*s * silu(g)) @ wo
a
Sharding: sequence-parallel. Core c handles batch c//4, tokens
[(c%4)*1024, (c%4+1)*1024). The cross-chunk scan carry is resolved with the
decomposition  s = s_local + cumA * s_in:  each core computes per-chunk
summaries (A_total, s_last), AllGathers them (8KB), combines prefixes
locally, and applies its incoming state as a per-channel scalar.

Schedule: the AllGather's completion is gated by the slowest core's arrival
(launch skew across the 8 PJRT devices is ~90us), so the kernel is ordered
to bank carry-independent work behind the collective: (k,v,a) projections +
local scans first -> single AllGather of both summary halves -> (g,q)
projections + wo prefetch while the gather is in flight -> only the carry
apply + output projection remain on the dependent tail.

gamma is folded into the five input-side projection weights on the host.
Matmuls run in fp16 at full (double-pumped) PE rate.
"""

import numpy as np

import concourse.bacc as bacc
import concourse.tile as tile
from concourse import mybir
from concourse.bass_utils import run_bass_kernel_spmd
from concourse.masks import make_identity

AFT = mybir.ActivationFunctionType
ALU = mybir.AluOpType
F32 = mybir.dt.float32
F16 = mybir.dt.float16

B, S, D = 2, 4096, 1024
NCORE = 8
GROUPS = 2              # batch groups of 4 cores
CPG = NCORE // GROUPS   # chunks (cores) per group
CHUNK = (B * S) // NCORE  # 1024 tokens per core
P = 128
NPT = D // P            # 8 channel ptiles
NM = CHUNK // P         # 8 token tiles
H = 512                 # psum half width (fp32 bank)
EPS = 1e-5
YSC = 4096.0            # fp16 range guard: y is carried as y/YSC
SW = 2 * NPT            # summary width: [A_total | s_last] per ptile

_CACHE = {}


def _build():
    nc = bacc.Bacc("TRN2", target_bir_lowering=False, debug=False,
                   num_devices=NCORE)
    x_in = nc.dram_tensor("x", [CHUNK, D], F32, kind="ExternalInput")
    w_in = {
        n: nc.dram_tensor(n, [NPT, P, D], F16, kind="ExternalInput")
        for n in ("wg", "wq", "wk", "wv", "wa")
    }
    w_in["wo"] = nc.dram_tensor("wo", [2, NPT, P, H], F16,
                                kind="ExternalInput")
    mask_in = nc.dram_tensor("mask", [P, NPT * NCORE], F32,
                             kind="ExternalInput")
    out_t = nc.dram_tensor("out", [CHUNK, D], F32, kind="ExternalOutput")

    with tc_ctx(nc) as tc:
        with (
            tc.tile_pool(name="const", bufs=1) as const,
            tc.tile_pool(name="xny", bufs=NPT) as xny,
            tc.tile_pool(name="persist", bufs=1) as persist,
            tc.tile_pool(name="scr", bufs=2) as scr,
            tc.tile_pool(name="tiny", bufs=4) as tiny,
            tc.tile_pool(name="wpool", bufs=4) as wpool,
            tc.tile_pool(name="wopool", bufs=16) as wopool,
            tc.tile_pool(name="small", bufs=1) as small,
            tc.tile_pool(name="dram", bufs=1, space="DRAM") as dram,
        ):
            ident = const.tile([P, P], F32)
            make_identity(nc, ident)

            maskt = small.tile([P, NPT, NCORE], F32)
            nc.sync.dma_start(
                out=maskt[:],
                in_=mask_in.rearrange("p (a b) -> p a b", a=NPT))

            xnT = [[persist.tile([P, H], F16, tag=f"xT{h}{d}",
                                 name=f"xnT{h}_{d}")
                    for d in range(NPT)] for h in range(2)]

            # ---- Phase A1: rmsnorm + transpose to [channel, token] ----
            # inv = rsqrt(ss+eps) via ACT Square-with-accum + a DVE Newton
            # iteration (fast-inverse-sqrt seed); sqrt(D) is folded into the
            # host-side weights, so no Sqrt table ever loads and the ACT
            # engine runs table-free through A1.
            MAGIC = 0xFFFFFFFF - 0x5F3759DF  # seed = ~((u>>1) + MAGIC)
            U32 = mybir.dt.uint32
            with tc.tile_pool(name="pst", bufs=4, space="PSUM") as pstp:
                for m in range(NM):
                    xm = scr.tile([P, D], F32, tag="x", name=f"x{m}")
                    nc.sync.dma_start(out=xm[:], in_=x_in[m * P:(m + 1) * P, :])
                    xn = scr.tile([P, D], F32, tag="xn", name=f"xn{m}")
                    ss = tiny.tile([P, 1], F32, tag="ss", name=f"ss{m}")
                    nc.scalar.activation(xn[:], xm[:], AFT.Square,
                                         accum_out=ss[:])
                    sv = tiny.tile([P, 1], F32, tag="sv", name=f"sv{m}")
                    nc.vector.tensor_scalar(
                        out=sv[:], in0=ss[:], scalar1=EPS, scalar2=None,
                        op0=ALU.add)
                    inv = tiny.tile([P, 1], F32, tag="inv", name=f"inv{m}")
                    nc.vector.tensor_single_scalar(
                        inv[:].bitcast(U32), sv[:].bitcast(U32), 1,
                        ALU.logical_shift_right)
                    nc.vector.tensor_single_scalar(
                        inv[:].bitcast(U32), inv[:].bitcast(U32), MAGIC,
                        ALU.add)
                    nc.vector.tensor_single_scalar(
                        inv[:].bitcast(U32), inv[:].bitcast(U32), 0,
                        ALU.bitwise_not)
                    t0 = tiny.tile([P, 1], F32, tag="t0", name=f"t0{m}")
                    for _ in range(2):
                        nc.vector.tensor_mul(t0[:], inv[:], inv[:])
                        nc.vector.tensor_mul(t0[:], t0[:], sv[:])
                        nc.vector.tensor_scalar(
                            out=t0[:], in0=t0[:], scalar1=-0.5, scalar2=1.5,
                            op0=ALU.mult, op1=ALU.add)
                        nc.vector.tensor_mul(inv[:], inv[:], t0[:])
                    nc.scalar.activation(xn[:], xm[:], AFT.Copy, scale=inv[:])
                    mh, mq = divmod(m, NM // 2)
                    for d in range(NPT):
                        pst = pstp.tile([P, P], F32, tag="pst",
                                        name=f"pst{m}_{d}")
                        nc.tensor.transpose(pst[:], xn[:, d * P:(d + 1) * P],
                                            ident[:])
                        nc.vector.tensor_copy(
                            xnT[mh][d][:, mq * P:(mq + 1) * P], pst[:])

            a_t = [persist.tile([P, CHUNK], F32, tag=f"a{p}", name=f"a{p}")
                   for p in range(NPT)]
            kv_t = [persist.tile([P, CHUNK], F32, tag=f"kv{p}", name=f"kv{p}")
                    for p in range(NPT)]
            sl_t = [persist.tile([P, CHUNK], F32, tag=f"sl{p}", name=f"sl{p}")
                    for p in range(NPT)]
            ca_t = [persist.tile([P, CHUNK], F32, tag=f"ca{p}", name=f"ca{p}")
                    for p in range(NPT)]
            summ = small.tile([P, SW], F32)

            # ---- Phase A2a: k,v,a projections + local scans + summaries ----
            with tc.tile_pool(name="psa", bufs=6, space="PSUM") as psa:
                for p in range(NPT):
                    ks = None
                    for wname, key in (("wk", "k"), ("wv", "v"), ("wa", "a")):
                        pts = [psa.tile([P, H], F32, tag="psa",
                                        name=f"ps_{key}{p}h{h}")
                               for h in range(2)]
                        wt = wpool.tile([P, D], F16, tag="w",
                                        name=f"w_{key}{p}")
                        nc.sync.dma_start(out=wt[:], in_=w_in[wname][p])
                        for h in range(2):
                            for k in range(NPT):
                                nc.tensor.matmul(
                                    pts[h][:],
                                    lhsT=wt[:, k * P:(k + 1) * P],
                                    rhs=xnT[h][k][:],
                                    start=(k == 0), stop=(k == NPT - 1))
                        for h in range(2):
                            hs = slice(h * H, (h + 1) * H)
                            if key == "k":
                                if ks is None:
                                    ks = [scr.tile([P, H], F32, tag="ks",
                                                   name=f"ks{p}h{hh}")
                                          for hh in range(2)]
                                nc.vector.tensor_copy(ks[h][:], pts[h][:])
                            elif key == "v":
                                nc.vector.tensor_mul(
                                    kv_t[p][:, hs], pts[h][:], ks[h][:])
                            elif key == "a":
                                nc.scalar.activation(a_t[p][:, hs],
                                                     pts[h][:], AFT.Sigmoid)
                    nc.vector.tensor_tensor_scan(
                        sl_t[p][:], a_t[p][:], kv_t[p][:], 0.0,
                        op0=ALU.mult, op1=ALU.add)
                    # A_total = prod(a) via mult-reduce; the cumulative-A
                    # scan itself is deferred into A2b where it overlaps
                    # the g/q matmuls instead of delaying the gather.
                    nc.vector.tensor_reduce(summ[:, p:p + 1], a_t[p][:],
                                            axis=mybir.AxisListType.X,
                                            op=ALU.mult)
                    nc.vector.tensor_copy(summ[:, NPT + p:NPT + p + 1],
                                          sl_t[p][:, CHUNK - 1:CHUNK])

            # ---- summary exchange: single AllGather, triggered ASAP ----
            import os as _os
            _nocc = bool(int(_os.environ.get("NOCC", "0")))
            cc_in = dram.tile([P, SW], F32, name="cc_in")
            cc_out = dram.tile([NCORE, P, SW], F32, addr_space="Shared",
                               name="cc_out")
            nc.gpsimd.dma_start(out=cc_in[:], in_=summ[:])
            if not _nocc:
                nc.gpsimd.collective_compute(
                    "AllGather", ALU.bypass,
                    replica_groups=[list(range(NCORE))],
                    ins=[cc_in[:]], outs=[cc_out[:]])

            # prefetch all output-projection weights while the gather flies
            woh = [[wopool.tile([P, H], F16, tag="woh", name=f"wo{h}k{k}")
                    for k in range(NPT)] for h in range(2)]
            for h in range(2):
                for k in range(NPT):
                    nc.sync.dma_start(out=woh[h][k][:], in_=w_in["wo"][h, k])

            # ---- Phase A2b: g,q projections; P = qg*sl, C = qg*ca ----
            with tc.tile_pool(name="psb", bufs=4, space="PSUM") as psb:
                for p in range(NPT):
                    gs = [scr.tile([P, H], F32, tag="gs", name=f"gs{p}h{hh}")
                          for hh in range(2)]
                    qg_p = scr.tile([P, CHUNK], F32, tag="qg", name=f"qg{p}")
                    for wname, key in (("wg", "g"), ("wq", "q")):
                        pts = [psb.tile([P, H], F32, tag="psb",
                                        name=f"ps_{key}{p}h{h}")
                               for h in range(2)]
                        wt = wpool.tile([P, D], F16, tag="w",
                                        name=f"w_{key}{p}")
                        nc.sync.dma_start(out=wt[:], in_=w_in[wname][p])
                        for h in range(2):
                            for k in range(NPT):
                                nc.tensor.matmul(
                                    pts[h][:],
                                    lhsT=wt[:, k * P:(k + 1) * P],
                                    rhs=xnT[h][k][:],
                                    start=(k == 0), stop=(k == NPT - 1))
                        for h in range(2):
                            if key == "g":
                                nc.scalar.activation(gs[h][:], pts[h][:],
                                                     AFT.Silu)
                            else:
                                nc.vector.scalar_tensor_tensor(
                                    out=qg_p[:, h * H:(h + 1) * H],
                                    in0=pts[h][:], scalar=1.0 / YSC,
                                    in1=gs[h][:],
                                    op0=ALU.mult, op1=ALU.mult)
                    nc.vector.tensor_tensor_scan(
                        ca_t[p][:], a_t[p][:], a_t[p][:], 1.0,
                        op0=ALU.mult, op1=ALU.bypass)
                    # P = qg * s_local (overwrites kv); C = qg * cumA
                    # (overwrites a)
                    nc.vector.tensor_mul(kv_t[p][:], qg_p[:], sl_t[p][:])
                    nc.vector.tensor_mul(a_t[p][:], qg_p[:], ca_t[p][:])

            # ---- gather consume + prefix combine ----
            gath = small.tile([P, NCORE * SW], F32)
            sin = small.tile([P, NPT], F32)
            if _nocc:
                nc.vector.memset(gath[:], 0.0)
            else:
                for c in range(NCORE):
                    nc.gpsimd.dma_start(
                        out=gath[:, c * SW:(c + 1) * SW], in_=cc_out[c])

            def A_of(j):
                return gath[:, j * SW: j * SW + NPT]

            def s_of(j):
                return gath[:, j * SW + NPT: j * SW + SW]

            cand = small.tile([P, NPT, NCORE], F32, name="cand")
            u = small.tile([P, NPT], F32, name="u")
            tmp = small.tile([P, NPT], F32, name="tmp")
            masked = small.tile([P, NPT, NCORE], F32, name="masked")
            y_t = [persist.tile([P, CHUNK], F16, tag=f"sl{p}",
                                name=f"y{p}") for p in range(NPT)]
            # Schedule the gather-gated combine/apply at the tail of the
            # vector queue: the Tile scheduler orders each engine's FIFO by
            # estimated ready time, and without the override it hoists these
            # (collective-gated) ops ahead of the remaining A2b evictions,
            # head-blocking the DVE queue behind the slowest core's arrival.
            with tc.tile_wait_until(1.0):
                nc.vector.memset(cand[:], 0.0)
                for g in range(GROUPS):
                    base = g * CPG
                    nc.vector.tensor_copy(u[:], s_of(base))
                    nc.vector.tensor_copy(cand[:, :, base + 1], u[:])
                    for jj in range(2, CPG):
                        nc.vector.tensor_mul(tmp[:], A_of(base + jj - 1),
                                             u[:])
                        nc.vector.tensor_add(u[:], tmp[:],
                                             s_of(base + jj - 1))
                        nc.vector.tensor_copy(cand[:, :, base + jj], u[:])
                nc.vector.tensor_mul(masked[:], cand[:], maskt[:])
                nc.vector.tensor_reduce(sin[:], masked[:],
                                        axis=mybir.AxisListType.X,
                                        op=ALU.add)

                # ---- Phase B: apply carry, output projection ----
                for m in range(NM):
                    ms = slice(m * P, (m + 1) * P)
                    for p in range(NPT):
                        nc.vector.scalar_tensor_tensor(
                            out=y_t[p][:, ms], in0=a_t[p][:, ms],
                            scalar=sin[:, p:p + 1], in1=kv_t[p][:, ms],
                            op0=ALU.mult, op1=ALU.add)

            with tc.tile_pool(name="pso", bufs=4, space="PSUM") as pso:
                for h in range(2):
                    for m in range(NM):
                        po = pso.tile([P, H], F32, tag="pso",
                                      name=f"po{h}m{m}")
                        for k in range(NPT):
                            nc.tensor.matmul(
                                po[:], lhsT=y_t[k][:, m * P:(m + 1) * P],
                                rhs=woh[h][k][:],
                                start=(k == 0), stop=(k == NPT - 1))
                        ostg = scr.tile([P, H], F32, tag="ostg",
                                        name=f"ostg{h}m{m}", bufs=2)
                        nc.scalar.activation(ostg[:], po[:], AFT.Copy)
                        nc.sync.dma_start(
                            out=out_t[m * P:(m + 1) * P, h * H:(h + 1) * H],
                            in_=ostg[:])

    nc.compile()
    return nc


def tc_ctx(nc):
    return tile.TileContext(nc)


def _get_nc():
    if "nc" not in _CACHE:
        _CACHE["nc"] = _build()
    return _CACHE["nc"]


def _blk_proj(w):
    # [din, dout] -> [p, r, (k, c)]: per dout-ptile slab, contiguous
    return np.ascontiguousarray(
        w.reshape(NPT, P, NPT, P).transpose(2, 1, 0, 3).reshape(NPT, P, D)
        .astype(np.float16))


def _blk_out(w):
    # [din, dout] -> [h, k, r, c]
    return np.ascontiguousarray(
        w.reshape(NPT, P, 2, H).transpose(2, 0, 1, 3).astype(np.float16))


def _make_in_maps(x, gamma, wq, wk, wv, wa, wg, wo):
    sg = gamma[:, None] * np.sqrt(float(D)).astype(np.float32)
    w_eff = {
        "wq": _blk_proj(sg * wq),
        "wk": _blk_proj(sg * wk),
        "wv": _blk_proj(sg * wv),
        "wa": _blk_proj(sg * wa),
        "wg": _blk_proj(sg * wg),
        "wo": _blk_out(wo * YSC),
    }
    in_maps = []
    for c in range(NCORE):
        b, ch = divmod(c, CPG)
        mask = np.zeros((P, NPT, NCORE), dtype=np.float32)
        mask[:, :, c] = 1.0
        in_maps.append({
            "x": np.ascontiguousarray(
                x[b, ch * CHUNK:(ch + 1) * CHUNK, :], dtype=np.float32),
            "mask": mask.reshape(P, NPT * NCORE),
            **w_eff,
        })
    return in_maps


def run_device(in_maps, trace=False, **kw):
    return run_bass_kernel_spmd(_get_nc(), in_maps, list(range(NCORE)),
                                trace=trace, **kw)


def _assemble(results):
    out = np.empty((B, S, D), dtype=np.float32)
    for c in range(NCORE):
        b, ch = divmod(c, CPG)
        out[b, ch * CHUNK:(ch + 1) * CHUNK, :] = results[c]["out"]
    return out


def kernel(x, gamma, wq, wk, wv, wa, wg, wo):
    in_maps = _make_in_maps(np.asarray(x), np.asarray(gamma), np.asarray(wq),
                            np.asarray(wk), np.asarray(wv), np.asarray(wa),
                            np.asarray(wg), np.asarray(wo))
    res = run_device(in_maps)
    return _assemble(res.results)
